# revision 18
# baseline (speedup 1.0000x reference)
"""Causal self-attention (k/q swapped variant) on 8 Trainium2 NeuronCores.

Problem (hardcoded shapes): B=2, N=2048, D=1024, H=16, DH=64.
  kqv = einsum('bnd,hde->bhne', x, Wkqv) + bkqv   ; split -> k, q, v
  A[b,h,n,m] = k[b,h,n]·q[b,h,m] / sqrt(DH), causal mask m<=n, softmax over m
  sa = A @ v ; concat heads ; out = sa @ Wo + bo

Sharding: tensor-parallel over heads — each core owns 2 heads (both batch
entries), computes its partial output projection sa_local @ Wo[rows], and the
host sums the 8 partials (+bo).

Per-core device kernel (all matmul operands bf16, fp32 PSUM accumulation):
  - x is pre-transposed on host to xt[b] = x[b].T ([D, N]) so the contraction
    dim d lands on SBUF partitions. Batch 0 is loaded in half-N pieces and
    its k/q projection runs d-chunk-major across 4 concurrent PSUM groups so
    the PE is paced by DMA arrival instead of stalling for the full 4MB.
  - k/q/v projections produce kT/qT/vT in [dh, n] layout with both heads
    stacked on the partition dim; biases are per-partition scalars there.
    v is then rotated to [n, dh] via PE transposes (stationary reuse beats
    the ldweights-bound direct [n, dh] projection).
  - scores are computed transposed, S^T[m, n] = q[m]·k[n], so softmax's
    reduction dim m sits on partitions; the denominator comes free from the
    PV matmul by augmenting v with 64 ones columns (the psum's other 64 rows
    hold the replicated row-sum). exp() is only computed on the causal
    region; the 128x128 diagonal triangle is zeroed with a 0/1 mask multiply.
  - output projection: stationary = saT column block, moving = Wo rows of the
    local heads -> natural-layout partial out [n, 1024] per block.
"""

import numpy as np
import ml_dtypes

B = 2
N = 2048
D = 1024
H = 16
DH = 64
NCORES = 8
HL = H // NCORES          # heads per core = 2
DC = D // 128             # contraction chunks = 8
NB = N // 128             # 128-row blocks = 16
NJ = N // 512             # 512-col blocks = 4

BF16 = ml_dtypes.bfloat16

_CACHE = {}


def _build():
    import concourse.bass as bass
    import concourse.mybir as mybir
    import concourse.tile as tile
    from concourse import bacc
    from contextlib import ExitStack

    f32 = mybir.dt.float32
    bf16 = mybir.dt.bfloat16

    nc = bacc.Bacc("TRN2", target_bir_lowering=False, debug=False,
                   enable_asserts=False, num_devices=NCORES)

    xt_d = nc.dram_tensor("xt", [B, D, N], bf16, kind="ExternalInput")
    wk_d = nc.dram_tensor("wk2", [D, 128], bf16, kind="ExternalInput")
    wq_d = nc.dram_tensor("wq2", [D, 128], bf16, kind="ExternalInput")
    wv_d = nc.dram_tensor("wv2", [D, 128], bf16, kind="ExternalInput")
    wo_d = nc.dram_tensor("wo2", [128, D], bf16, kind="ExternalInput")
    bk_d = nc.dram_tensor("bk2", [128, 1], f32, kind="ExternalInput")
    bq_d = nc.dram_tensor("bq2", [128, 1], f32, kind="ExternalInput")
    bv_d = nc.dram_tensor("bv2", [128, 1], f32, kind="ExternalInput")
    eye_d = nc.dram_tensor("eye2", [128, 64], bf16, kind="ExternalInput")
    m01_d = nc.dram_tensor("m01", [128, 128], bf16, kind="ExternalInput")
    out_d = nc.dram_tensor("out", [B, N, D], f32, kind="ExternalOutput")

    with tile.TileContext(nc) as tc, ExitStack() as ctx:
        const = ctx.enter_context(tc.tile_pool(name="const", bufs=1))
        xt_pool = ctx.enter_context(tc.tile_pool(name="xt", bufs=1))
        kq_pool = ctx.enter_context(tc.tile_pool(name="kq", bufs=6))
        v_pool = ctx.enter_context(tc.tile_pool(name="v", bufs=2))
        sa_pool = ctx.enter_context(tc.tile_pool(name="sa", bufs=2))
        pt_pool = ctx.enter_context(tc.tile_pool(name="pt", bufs=4))
        rc_pool = ctx.enter_context(tc.tile_pool(name="rc", bufs=2))
        ob_pool = ctx.enter_context(tc.tile_pool(name="ob", bufs=4))
        proj_ps = ctx.enter_context(tc.tile_pool(name="proj_ps", bufs=2, space="PSUM"))
        s_ps = ctx.enter_context(tc.tile_pool(name="s_ps", bufs=2, space="PSUM"))
        pv_ps = ctx.enter_context(tc.tile_pool(name="pv_ps", bufs=2, space="PSUM"))
        out_ps = ctx.enter_context(tc.tile_pool(name="out_ps", bufs=2, space="PSUM"))

        # ---- DMA issue order: k/q weights -> xt batch 0 (half-N pieces,
        # alternating HWDGE rings) -> remaining consts -> xt batch 1.
        xt0 = {}   # (dc, half) -> [128, 1024]
        xt1 = {}   # dc -> [128, 2048]

        wk_sb = const.tile([128, DC * 128], bf16, name="wk_sb")
        wq_sb = const.tile([128, DC * 128], bf16, name="wq_sb")
        wv_sb = const.tile([128, DC * 128], bf16, name="wv_sb")
        for w_sb, w_d in ((wk_sb, wk_d), (wq_sb, wq_d)):
            nc.sync.dma_start(
                w_sb[:].rearrange("p (dc m) -> p dc m", dc=DC),
                w_d.ap().rearrange("(dc p) m -> p dc m", p=128))
        for half in range(2):
            for dc in range(DC):
                t = xt_pool.tile([128, 1024], bf16, name=f"xt0_{dc}_{half}",
                                 tag="xt0", bufs=2 * DC)
                eng = nc.sync if dc % 2 == 0 else nc.scalar
                eng.dma_start(t[:], xt_d.ap()[0, dc * 128:(dc + 1) * 128,
                                              half * 1024:(half + 1) * 1024])
                xt0[dc, half] = t
        nc.sync.dma_start(
            wv_sb[:].rearrange("p (dc m) -> p dc m", dc=DC),
            wv_d.ap().rearrange("(dc p) m -> p dc m", p=128))
        wo_sb = const.tile([128, D], bf16, name="wo_sb")
        nc.sync.dma_start(wo_sb[:], wo_d.ap())
        bk_sb = const.tile([128, 1], f32, name="bk_sb")
        nc.sync.dma_start(bk_sb[:], bk_d.ap())
        bq_sb = const.tile([128, 1], f32, name="bq_sb")
        nc.sync.dma_start(bq_sb[:], bq_d.ap())
        bv_sb = const.tile([128, 1], f32, name="bv_sb")
        nc.sync.dma_start(bv_sb[:], bv_d.ap())
        eye_sb = const.tile([128, 64], bf16, name="eye_sb")
        nc.sync.dma_start(eye_sb[:], eye_d.ap())
        m01_sb = const.tile([128, 128], bf16, name="m01_sb")
        nc.sync.dma_start(m01_sb[:], m01_d.ap())
        for dc in range(DC):
            t = xt_pool.tile([128, N], bf16, name=f"xt1_{dc}", tag="xt1",
                             bufs=DC)
            nc.sync.dma_start(t[:], xt_d.ap()[1, dc * 128:(dc + 1) * 128, :])
            xt1[dc] = t

        def xt_ap(b, dc, c0, c1):
            if b == 1:
                return xt1[dc][:, c0:c1]
            half = c0 // 1024
            return xt0[dc, half][:, c0 - half * 1024:c1 - half * 1024]

        for b in range(B):
            # ---- k / q projections -> [128(2 heads x dh), N] bf16
            k2_sb = kq_pool.tile([128, N], bf16, name=f"k2_b{b}", tag="kq")
            q2_sb = kq_pool.tile([128, N], bf16, name=f"q2_b{b}", tag="kq")
            groups = ((wk_sb, bk_sb, k2_sb), (wq_sb, bq_sb, q2_sb))
            if b == 0:
                # d-chunk-major across 4 concurrent psum groups, paced by the
                # half-N piece DMAs (borrows the idle out_ps pool's banks)
                for half in range(2):
                    pss = {}
                    for gi in range(2):
                        for njl in range(2):
                            pool, tag = ((proj_ps, "proj") if gi == 0
                                         else (out_ps, "op"))
                            pss[gi, njl] = pool.tile([128, 512], f32,
                                                     name="kq_ps", tag=tag)
                    for dc in range(DC):
                        for gi, (w_sb, _, _) in enumerate(groups):
                            for njl in range(2):
                                nc.tensor.matmul(
                                    pss[gi, njl][:],
                                    w_sb[:, dc * 128:(dc + 1) * 128],
                                    xt0[dc, half][:, njl * 512:(njl + 1) * 512],
                                    start=(dc == 0), stop=(dc == DC - 1))
                    for gi, (_, bias_sb, dst) in enumerate(groups):
                        for njl in range(2):
                            nj = 2 * half + njl
                            nc.vector.tensor_scalar_add(
                                dst[:, nj * 512:(nj + 1) * 512],
                                pss[gi, njl][:], bias_sb[:])
            else:
                for w_sb, bias_sb, dst in groups:
                    for nj in range(NJ):
                        ps = proj_ps.tile([128, 512], f32, name="kq_ps",
                                          tag="proj")
                        for dc in range(DC):
                            nc.tensor.matmul(
                                ps[:], w_sb[:, dc * 128:(dc + 1) * 128],
                                xt_ap(b, dc, nj * 512, (nj + 1) * 512),
                                start=(dc == 0), stop=(dc == DC - 1))
                        nc.vector.tensor_scalar_add(
                            dst[:, nj * 512:(nj + 1) * 512], ps[:], bias_sb[:])

            # ---- v projection: vT [dh-stacked, N] (stationary-reusing),
            # bias as per-partition scalar, then PE-transpose into
            # [n, 192-blocks: v_h0 | ones | v_h1] bf16
            vt_sb = kq_pool.tile([128, N], bf16, name=f"vt_b{b}", tag="kq")
            for nj in range(NJ):
                ps = proj_ps.tile([128, 512], f32, name="vt_ps", tag="proj")
                for dc in range(DC):
                    nc.tensor.matmul(
                        ps[:], wv_sb[:, dc * 128:(dc + 1) * 128],
                        xt_ap(b, dc, nj * 512, (nj + 1) * 512),
                        start=(dc == 0), stop=(dc == DC - 1))
                nc.vector.tensor_scalar_add(
                    vt_sb[:, nj * 512:(nj + 1) * 512], ps[:], bv_sb[:])
            v_sb = v_pool.tile([128, NB * 192], bf16, name=f"v_b{b}", tag="v")
            nc.vector.memset(
                v_sb[:].rearrange("p (nb g) -> p nb g", g=192)[:, :, 64:128], 1.0)
            for nb in range(NB):
                for h in range(HL):
                    hp = 64 * h
                    tp = proj_ps.tile([128, 64], bf16, name="tp", tag="proj")
                    nc.tensor.transpose(
                        tp[:], vt_sb[hp:hp + 64, nb * 128:(nb + 1) * 128],
                        eye_sb[hp:hp + 64, :])
                    nc.vector.tensor_copy(
                        v_sb[:, nb * 192 + 128 * h:nb * 192 + 128 * h + 64],
                        tp[:])

            # ---- attention
            sa_sb = sa_pool.tile([128, N], bf16, name=f"sa_b{b}", tag="sa")
            for j in range(NJ):
                pv = [pv_ps.tile([128, 512], f32, name=f"pv{h}", tag="pv")
                      for h in range(HL)]
                nch = 4 * (j + 1)
                for ci in range(nch):
                    t = ci - 4 * j
                    lo = 128 * t if t >= 0 else 0
                    for h in range(HL):
                        hp = 64 * h
                        sp = s_ps.tile([128, 512], f32, name="s", tag="s")
                        nc.tensor.matmul(
                            sp[:, lo:512],
                            q2_sb[hp:hp + 64, ci * 128:(ci + 1) * 128],
                            k2_sb[hp:hp + 64, j * 512 + lo:(j + 1) * 512],
                            start=True, stop=True)
                        pt = pt_pool.tile([128, 512], bf16, name="pt", tag="pt")
                        nc.scalar.activation(
                            pt[:, lo:512], sp[:, lo:512],
                            mybir.ActivationFunctionType.Exp, scale=0.125)
                        if t >= 0:
                            nc.vector.tensor_tensor(
                                pt[:, lo:lo + 128], pt[:, lo:lo + 128],
                                m01_sb[:], mybir.AluOpType.mult)
                        nc.tensor.matmul(
                            pv[h][:, lo:512],
                            v_sb[:, ci * 192 + 64 * h:ci * 192 + 64 * h + 128],
                            pt[:, lo:512],
                            start=(ci == 0), stop=(ci == nch - 1))
                for h in range(HL):
                    # h0: rows 0:64 = sa, 64:128 = denom ; h1: swapped
                    sa_rows = pv[h][64 * h:64 * h + 64, :]
                    den_rows = pv[h][64 - 64 * h:128 - 64 * h, :]
                    # denominators are sums of exp() in [~2e-3, ~3e3]: safely
                    # inside approx_fast's domain; its 18-bit accuracy is far
                    # below the bf16 noise of the P*V numerator. (approx_fast
                    # misreads PSUM operands on HW - bounce through SBUF.)
                    den_sb = rc_pool.tile([64, 512], f32, name="den", tag="den")
                    nc.vector.tensor_copy(den_sb[:], den_rows)
                    rc = rc_pool.tile([64, 512], f32, name="rc", tag="rc")
                    nc.vector.reciprocal_approx_fast(rc[:], den_sb[:])
                    nc.vector.tensor_tensor(
                        sa_sb[64 * h:64 * h + 64, j * 512:(j + 1) * 512],
                        sa_rows, rc[:], mybir.AluOpType.mult)

            # ---- output projection (partial over local heads)
            for nb in range(NB):
                for half in range(2):
                    op = out_ps.tile([128, 512], f32, name="op", tag="op")
                    nc.tensor.matmul(
                        op[:], sa_sb[:, nb * 128:(nb + 1) * 128],
                        wo_sb[:, half * 512:(half + 1) * 512],
                        start=True, stop=True)
                    ob = ob_pool.tile([128, 512], f32, name="ob", tag="ob")
                    if half == 0:
                        nc.scalar.copy(ob[:], op[:])
                    else:
                        nc.vector.tensor_copy(ob[:], op[:])
                    nc.sync.dma_start(
                        out_d.ap()[b, nb * 128:(nb + 1) * 128,
                                   half * 512:(half + 1) * 512], ob[:])

    nc.compile()
    return nc


def _get_nc():
    if "nc" not in _CACHE:
        _CACHE["nc"] = _build()
    return _CACHE["nc"]


def _prep_inputs(x, Wkqv, bkqv, Wo, bo):
    """Host-side shard prep: one input map per core."""
    xt = np.ascontiguousarray(x.transpose(0, 2, 1)).astype(BF16)
    tri = np.triu(np.ones((128, 128), np.float32)).astype(BF16)  # m' <= n''
    eye2 = np.concatenate([np.eye(64, dtype=np.float32)] * 2, axis=0).astype(BF16)
    in_maps = []
    for c in range(NCORES):
        h0, h1 = HL * c, HL * c + 1
        wk2 = np.concatenate([Wkqv[h0, :, 0:64], Wkqv[h1, :, 0:64]], axis=1)
        wq2 = np.concatenate([Wkqv[h0, :, 64:128], Wkqv[h1, :, 64:128]], axis=1)
        wv2 = np.concatenate([Wkqv[h0, :, 128:192], Wkqv[h1, :, 128:192]], axis=1)
        bk2 = np.concatenate([bkqv[h0, 0:64], bkqv[h1, 0:64]])[:, None]
        bq2 = np.concatenate([bkqv[h0, 64:128], bkqv[h1, 64:128]])[:, None]
        bv2 = np.concatenate([bkqv[h0, 128:192], bkqv[h1, 128:192]])[:, None]
        in_maps.append({
            "xt": xt,
            "wk2": wk2.astype(BF16),
            "wq2": wq2.astype(BF16),
            "wv2": wv2.astype(BF16),
            "wo2": Wo[128 * c:128 * (c + 1), :].astype(BF16),
            "bk2": np.ascontiguousarray(bk2, np.float32),
            "bq2": np.ascontiguousarray(bq2, np.float32),
            "bv2": np.ascontiguousarray(bv2, np.float32),
            "eye2": eye2,
            "m01": tri,
        })
    return in_maps


def kernel(x, Wkqv, bkqv, Wo, bo):
    from concourse import bass_utils

    nc = _get_nc()
    in_maps = _prep_inputs(np.asarray(x), np.asarray(Wkqv), np.asarray(bkqv),
                           np.asarray(Wo), np.asarray(bo))
    res = bass_utils.run_bass_kernel_spmd(nc, in_maps, core_ids=list(range(NCORES)))
    acc = np.zeros((B, N, D), np.float32)
    for c in range(NCORES):
        acc += res.results[c]["out"]
    acc += np.asarray(bo)[None, None, :]
    return acc


# revision 19
# speedup vs baseline: 1.0484x; 1.0484x over previous
"""Causal self-attention (k/q swapped variant) on 8 Trainium2 NeuronCores.

Problem (hardcoded shapes): B=2, N=2048, D=1024, H=16, DH=64.
  kqv = einsum('bnd,hde->bhne', x, Wkqv) + bkqv   ; split -> k, q, v
  A[b,h,n,m] = k[b,h,n]·q[b,h,m] / sqrt(DH), causal mask m<=n, softmax over m
  sa = A @ v ; concat heads ; out = sa @ Wo + bo

Sharding: tensor-parallel over heads — each core owns 2 heads (both batch
entries), computes its partial output projection sa_local @ Wo[rows], and the
host sums the 8 partials (+bo).

Per-core device kernel (all matmul operands bf16, fp32 PSUM accumulation):
  - x is pre-transposed on host to xt[b] = x[b].T ([D, N]) so the contraction
    dim d lands on SBUF partitions. Batch 0 is loaded in half-N pieces and
    its k/q projection runs d-chunk-major across 4 concurrent PSUM groups so
    the PE is paced by DMA arrival instead of stalling for the full 4MB.
  - k/q/v projections produce kT/qT/vT in [dh, n] layout with both heads
    stacked on the partition dim; biases are per-partition scalars there.
    v is then rotated to [n, dh] via PE transposes (stationary reuse beats
    the ldweights-bound direct [n, dh] projection).
  - scores are computed transposed, S^T[m, n] = q[m]·k[n], so softmax's
    reduction dim m sits on partitions; the denominator comes free from the
    PV matmul by augmenting v with 64 ones columns (the psum's other 64 rows
    hold the replicated row-sum). exp() is only computed on the causal
    region; the 128x128 diagonal triangle is zeroed with a 0/1 mask multiply.
  - output projection: stationary = saT column block, moving = Wo rows of the
    local heads -> natural-layout partial out [n, 1024] per block.
"""

import numpy as np
import ml_dtypes

B = 2
N = 2048
D = 1024
H = 16
DH = 64
NCORES = 8
HL = H // NCORES          # heads per core = 2
DC = D // 128             # contraction chunks = 8
NB = N // 128             # 128-row blocks = 16
NJ = N // 512             # 512-col blocks = 4

BF16 = ml_dtypes.bfloat16

_CACHE = {}


def _build():
    import concourse.bass as bass
    import concourse.mybir as mybir
    import concourse.tile as tile
    from concourse import bacc
    from contextlib import ExitStack

    f32 = mybir.dt.float32
    bf16 = mybir.dt.bfloat16

    nc = bacc.Bacc("TRN2", target_bir_lowering=False, debug=False,
                   enable_asserts=False, num_devices=NCORES)

    xt_d = nc.dram_tensor("xt", [B, D, N], bf16, kind="ExternalInput")
    wk_d = nc.dram_tensor("wk2", [D, 128], bf16, kind="ExternalInput")
    wq_d = nc.dram_tensor("wq2", [D, 128], bf16, kind="ExternalInput")
    wv_d = nc.dram_tensor("wv2", [D, 128], bf16, kind="ExternalInput")
    wo_d = nc.dram_tensor("wo2", [128, D], bf16, kind="ExternalInput")
    bk_d = nc.dram_tensor("bk2", [128, 1], f32, kind="ExternalInput")
    bq_d = nc.dram_tensor("bq2", [128, 1], f32, kind="ExternalInput")
    bv_d = nc.dram_tensor("bv2", [128, 1], f32, kind="ExternalInput")
    eye_d = nc.dram_tensor("eye2", [128, 64], bf16, kind="ExternalInput")
    m01_d = nc.dram_tensor("m01", [128, 128], bf16, kind="ExternalInput")
    out_d = nc.dram_tensor("out", [B, N, D], f32, kind="ExternalOutput")

    with tile.TileContext(nc) as tc, ExitStack() as ctx:
        const = ctx.enter_context(tc.tile_pool(name="const", bufs=1))
        xt_pool = ctx.enter_context(tc.tile_pool(name="xt", bufs=1))
        kq_pool = ctx.enter_context(tc.tile_pool(name="kq", bufs=6))
        v_pool = ctx.enter_context(tc.tile_pool(name="v", bufs=2))
        sa_pool = ctx.enter_context(tc.tile_pool(name="sa", bufs=2))
        pt_pool = ctx.enter_context(tc.tile_pool(name="pt", bufs=4))
        rc_pool = ctx.enter_context(tc.tile_pool(name="rc", bufs=2))
        ob_pool = ctx.enter_context(tc.tile_pool(name="ob", bufs=4))
        proj_ps = ctx.enter_context(tc.tile_pool(name="proj_ps", bufs=2, space="PSUM"))
        s_ps = ctx.enter_context(tc.tile_pool(name="s_ps", bufs=2, space="PSUM"))
        pv_ps = ctx.enter_context(tc.tile_pool(name="pv_ps", bufs=2, space="PSUM"))
        out_ps = ctx.enter_context(tc.tile_pool(name="out_ps", bufs=2, space="PSUM"))

        # ---- DMA issue order: k/q weights -> xt batch 0 (half-N pieces,
        # alternating HWDGE rings) -> remaining consts -> xt batch 1.
        xt0 = {}   # (dc, half) -> [128, 1024]
        xt1 = {}   # dc -> [128, 2048]

        wk_sb = const.tile([128, DC * 128], bf16, name="wk_sb")
        wq_sb = const.tile([128, DC * 128], bf16, name="wq_sb")
        wv_sb = const.tile([128, DC * 128], bf16, name="wv_sb")
        for w_sb, w_d in ((wk_sb, wk_d), (wq_sb, wq_d)):
            nc.sync.dma_start(
                w_sb[:].rearrange("p (dc m) -> p dc m", dc=DC),
                w_d.ap().rearrange("(dc p) m -> p dc m", p=128))
        for half in range(2):
            for dc in range(DC):
                t = xt_pool.tile([128, 1024], bf16, name=f"xt0_{dc}_{half}",
                                 tag="xt0", bufs=2 * DC)
                eng = nc.sync if dc % 2 == 0 else nc.scalar
                eng.dma_start(t[:], xt_d.ap()[0, dc * 128:(dc + 1) * 128,
                                              half * 1024:(half + 1) * 1024])
                xt0[dc, half] = t
        nc.sync.dma_start(
            wv_sb[:].rearrange("p (dc m) -> p dc m", dc=DC),
            wv_d.ap().rearrange("(dc p) m -> p dc m", p=128))
        wo_sb = const.tile([128, D], bf16, name="wo_sb")
        nc.sync.dma_start(wo_sb[:], wo_d.ap())
        bk_sb = const.tile([128, 1], f32, name="bk_sb")
        nc.sync.dma_start(bk_sb[:], bk_d.ap())
        bq_sb = const.tile([128, 1], f32, name="bq_sb")
        nc.sync.dma_start(bq_sb[:], bq_d.ap())
        bv_sb = const.tile([128, 1], f32, name="bv_sb")
        nc.sync.dma_start(bv_sb[:], bv_d.ap())
        eye_sb = const.tile([128, 64], bf16, name="eye_sb")
        nc.sync.dma_start(eye_sb[:], eye_d.ap())
        m01_sb = const.tile([128, 128], bf16, name="m01_sb")
        nc.sync.dma_start(m01_sb[:], m01_d.ap())
        for dc in range(DC):
            t = xt_pool.tile([128, N], bf16, name=f"xt1_{dc}", tag="xt1",
                             bufs=DC)
            nc.sync.dma_start(t[:], xt_d.ap()[1, dc * 128:(dc + 1) * 128, :])
            xt1[dc] = t

        def xt_ap(b, dc, c0, c1):
            if b == 1:
                return xt1[dc][:, c0:c1]
            half = c0 // 1024
            return xt0[dc, half][:, c0 - half * 1024:c1 - half * 1024]

        for b in range(B):
            # ---- k / q projections -> [128(2 heads x dh), N] bf16
            k2_sb = kq_pool.tile([128, N], bf16, name=f"k2_b{b}", tag="kq")
            q2_sb = kq_pool.tile([128, N], bf16, name=f"q2_b{b}", tag="kq")
            groups = ((wk_sb, bk_sb, k2_sb), (wq_sb, bq_sb, q2_sb))
            if b == 0:
                # d-chunk-major across 4 concurrent psum groups, paced by the
                # half-N piece DMAs (borrows the idle out_ps pool's banks)
                for half in range(2):
                    pss = {}
                    for gi in range(2):
                        for njl in range(2):
                            pool, tag = ((proj_ps, "proj") if gi == 0
                                         else (out_ps, "op"))
                            pss[gi, njl] = pool.tile([128, 512], f32,
                                                     name="kq_ps", tag=tag)
                    for dc in range(DC):
                        for gi, (w_sb, _, _) in enumerate(groups):
                            for njl in range(2):
                                nc.tensor.matmul(
                                    pss[gi, njl][:],
                                    w_sb[:, dc * 128:(dc + 1) * 128],
                                    xt0[dc, half][:, njl * 512:(njl + 1) * 512],
                                    start=(dc == 0), stop=(dc == DC - 1))
                    for gi, (_, bias_sb, dst) in enumerate(groups):
                        for njl in range(2):
                            nj = 2 * half + njl
                            nc.vector.tensor_scalar_add(
                                dst[:, nj * 512:(nj + 1) * 512],
                                pss[gi, njl][:], bias_sb[:])
            else:
                for w_sb, bias_sb, dst in groups:
                    for nj in range(NJ):
                        ps = proj_ps.tile([128, 512], f32, name="kq_ps",
                                          tag="proj")
                        for dc in range(DC):
                            nc.tensor.matmul(
                                ps[:], w_sb[:, dc * 128:(dc + 1) * 128],
                                xt_ap(b, dc, nj * 512, (nj + 1) * 512),
                                start=(dc == 0), stop=(dc == DC - 1))
                        nc.vector.tensor_scalar_add(
                            dst[:, nj * 512:(nj + 1) * 512], ps[:], bias_sb[:])

            # ---- v projection: vT [dh-stacked, N] (stationary-reusing),
            # bias as per-partition scalar, then PE-transpose into
            # [n, 192-blocks: v_h0 | ones | v_h1] bf16
            vt_sb = kq_pool.tile([128, N], bf16, name=f"vt_b{b}", tag="kq")
            for nj in range(NJ):
                ps = proj_ps.tile([128, 512], f32, name="vt_ps", tag="proj")
                for dc in range(DC):
                    nc.tensor.matmul(
                        ps[:], wv_sb[:, dc * 128:(dc + 1) * 128],
                        xt_ap(b, dc, nj * 512, (nj + 1) * 512),
                        start=(dc == 0), stop=(dc == DC - 1))
                nc.vector.tensor_scalar_add(
                    vt_sb[:, nj * 512:(nj + 1) * 512], ps[:], bv_sb[:])
            v_sb = v_pool.tile([128, NB * 192], bf16, name=f"v_b{b}", tag="v")
            nc.vector.memset(
                v_sb[:].rearrange("p (nb g) -> p nb g", g=192)[:, :, 64:128], 1.0)

            def transpose_v(nb):
                # rotate vT[dh, n] -> v[n, dh] for one 128-row chunk; emitted
                # interleaved with the attention stream (not as its own phase)
                # so the PE activity monitor never sees a sparse stretch.
                for h in range(HL):
                    hp = 64 * h
                    tp = proj_ps.tile([128, 64], bf16, name="tp", tag="proj")
                    nc.tensor.transpose(
                        tp[:], vt_sb[hp:hp + 64, nb * 128:(nb + 1) * 128],
                        eye_sb[hp:hp + 64, :])
                    nc.vector.tensor_copy(
                        v_sb[:, nb * 192 + 128 * h:nb * 192 + 128 * h + 64],
                        tp[:])

            # ---- attention
            sa_sb = sa_pool.tile([128, N], bf16, name=f"sa_b{b}", tag="sa")
            for j in range(NJ):
                for nb in range(4 * j, 4 * j + 4):
                    transpose_v(nb)
                pv = [pv_ps.tile([128, 512], f32, name=f"pv{h}", tag="pv")
                      for h in range(HL)]
                nch = 4 * (j + 1)
                for ci in range(nch):
                    t = ci - 4 * j
                    lo = 128 * t if t >= 0 else 0
                    for h in range(HL):
                        hp = 64 * h
                        sp = s_ps.tile([128, 512], f32, name="s", tag="s")
                        nc.tensor.matmul(
                            sp[:, lo:512],
                            q2_sb[hp:hp + 64, ci * 128:(ci + 1) * 128],
                            k2_sb[hp:hp + 64, j * 512 + lo:(j + 1) * 512],
                            start=True, stop=True)
                        pt = pt_pool.tile([128, 512], bf16, name="pt", tag="pt")
                        nc.scalar.activation(
                            pt[:, lo:512], sp[:, lo:512],
                            mybir.ActivationFunctionType.Exp, scale=0.125)
                        if t >= 0:
                            nc.vector.tensor_tensor(
                                pt[:, lo:lo + 128], pt[:, lo:lo + 128],
                                m01_sb[:], mybir.AluOpType.mult)
                        nc.tensor.matmul(
                            pv[h][:, lo:512],
                            v_sb[:, ci * 192 + 64 * h:ci * 192 + 64 * h + 128],
                            pt[:, lo:512],
                            start=(ci == 0), stop=(ci == nch - 1))
                for h in range(HL):
                    # h0: rows 0:64 = sa, 64:128 = denom ; h1: swapped
                    sa_rows = pv[h][64 * h:64 * h + 64, :]
                    den_rows = pv[h][64 - 64 * h:128 - 64 * h, :]
                    # denominators are sums of exp() in [~2e-3, ~3e3]: safely
                    # inside approx_fast's domain; its 18-bit accuracy is far
                    # below the bf16 noise of the P*V numerator. (approx_fast
                    # misreads PSUM operands on HW - bounce through SBUF.)
                    den_sb = rc_pool.tile([64, 512], f32, name="den", tag="den")
                    nc.vector.tensor_copy(den_sb[:], den_rows)
                    rc = rc_pool.tile([64, 512], f32, name="rc", tag="rc")
                    nc.vector.reciprocal_approx_fast(rc[:], den_sb[:])
                    nc.vector.tensor_tensor(
                        sa_sb[64 * h:64 * h + 64, j * 512:(j + 1) * 512],
                        sa_rows, rc[:], mybir.AluOpType.mult)

            # ---- output projection (partial over local heads)
            for nb in range(NB):
                for half in range(2):
                    op = out_ps.tile([128, 512], f32, name="op", tag="op")
                    nc.tensor.matmul(
                        op[:], sa_sb[:, nb * 128:(nb + 1) * 128],
                        wo_sb[:, half * 512:(half + 1) * 512],
                        start=True, stop=True)
                    ob = ob_pool.tile([128, 512], f32, name="ob", tag="ob")
                    if half == 0:
                        nc.scalar.copy(ob[:], op[:])
                    else:
                        nc.vector.tensor_copy(ob[:], op[:])
                    nc.sync.dma_start(
                        out_d.ap()[b, nb * 128:(nb + 1) * 128,
                                   half * 512:(half + 1) * 512], ob[:])

    nc.compile()
    return nc


def _get_nc():
    if "nc" not in _CACHE:
        _CACHE["nc"] = _build()
    return _CACHE["nc"]


def _prep_inputs(x, Wkqv, bkqv, Wo, bo):
    """Host-side shard prep: one input map per core."""
    xt = np.ascontiguousarray(x.transpose(0, 2, 1)).astype(BF16)
    tri = np.triu(np.ones((128, 128), np.float32)).astype(BF16)  # m' <= n''
    eye2 = np.concatenate([np.eye(64, dtype=np.float32)] * 2, axis=0).astype(BF16)
    in_maps = []
    for c in range(NCORES):
        h0, h1 = HL * c, HL * c + 1
        wk2 = np.concatenate([Wkqv[h0, :, 0:64], Wkqv[h1, :, 0:64]], axis=1)
        wq2 = np.concatenate([Wkqv[h0, :, 64:128], Wkqv[h1, :, 64:128]], axis=1)
        wv2 = np.concatenate([Wkqv[h0, :, 128:192], Wkqv[h1, :, 128:192]], axis=1)
        bk2 = np.concatenate([bkqv[h0, 0:64], bkqv[h1, 0:64]])[:, None]
        bq2 = np.concatenate([bkqv[h0, 64:128], bkqv[h1, 64:128]])[:, None]
        bv2 = np.concatenate([bkqv[h0, 128:192], bkqv[h1, 128:192]])[:, None]
        in_maps.append({
            "xt": xt,
            "wk2": wk2.astype(BF16),
            "wq2": wq2.astype(BF16),
            "wv2": wv2.astype(BF16),
            "wo2": Wo[128 * c:128 * (c + 1), :].astype(BF16),
            "bk2": np.ascontiguousarray(bk2, np.float32),
            "bq2": np.ascontiguousarray(bq2, np.float32),
            "bv2": np.ascontiguousarray(bv2, np.float32),
            "eye2": eye2,
            "m01": tri,
        })
    return in_maps


def kernel(x, Wkqv, bkqv, Wo, bo):
    from concourse import bass_utils

    nc = _get_nc()
    in_maps = _prep_inputs(np.asarray(x), np.asarray(Wkqv), np.asarray(bkqv),
                           np.asarray(Wo), np.asarray(bo))
    res = bass_utils.run_bass_kernel_spmd(nc, in_maps, core_ids=list(range(NCORES)))
    acc = np.zeros((B, N, D), np.float32)
    for c in range(NCORES):
        acc += res.results[c]["out"]
    acc += np.asarray(bo)[None, None, :]
    return acc
